# revision 25
# baseline (speedup 1.0000x reference)
"""DigitCaps dynamic-routing kernel for 8 TRN2 NeuronCores.

Algorithm (never materializes u_hat):
  Shard over capsules C=96 -> 12 per core (makes every routing step
  core-local: softmax over R is per-capsule, the batch-mean a_ij needs
  no cross-core reduction -> zero collectives).

  Per core, with K = R*I = 3840 the flattened contraction dim:

  s-phase (k-layout A: k = i*2*96 blocks, chunk t <-> i=t//2, r=(t%2)*96+p):
    s[b,(c,o)]    = sum_k wc[k,(c,o)] * xt[k, b]           (PE, 80 mm)
    wc[k,(c,o)]   = cB[k,(c,o)] * wt[k,(c,o)]              (DVE bf16 2x)
    v = squash(s)                                          (ACT+DVE, small)

  a-phase (k-layout B: k = r*20 + i, (c,o) on partitions):
    M'[(c,o), k]  = sum_b v[b,(c,o)] * x[b, k]             (PE, 32 mm N=512,
                                                            stationary=v reused)
    P = W2 .* M'  (ACT copies psum->bf16, DVE 2x muls)
    Q[(c,o), r]   = sum_i P                                (DVE reduce, step-1)
    a[c, r]       = SEL.T @ Q  (SEL = delta(c)/B)          (PE, tiny)
    b_ij += a; c_ij = softmax_r(b_ij)  (two [6,192] half-tiles)

  Matmul inputs bf16 (measured ~3.8e-3 global rel err vs f32 reference),
  routing/softmax/squash math in f32.
"""

import numpy as np
import ml_dtypes

import concourse.bass as bass
import concourse.mybir as mybir
from concourse import tile
from concourse.vector_clock import ScopedClock

B, R, C, O, I = 256, 192, 96, 16, 20
NCORES = 8
CL = C // NCORES          # 12 capsules per core
M = CL * O                # 192 = (c,o) free dim per core
KC = R // 2               # 96 = (c,o) rows per half (a-phase partition dim)
KT = R * I                # 3840 total contraction
KP = 128                  # s-phase k-chunk depth (full PE rows)
NK = KT // KP             # 30 s-phase k-chunks
NW = 2                    # wx SBUF tiles (15 chunks each)
CPW = NK // NW            # 15
NJ = 8                    # a-phase moving slices (3840 = 8 * 480)
JW = KT // NJ             # 480
NITER = 3

F32 = mybir.dt.float32
BF16 = mybir.dt.bfloat16
AF = mybir.ActivationFunctionType
ALU = mybir.AluOpType
AX = mybir.AxisListType


class _TC(tile.TileContext):
    """TileContext whose exit drain splits its semaphore waits across
    chained SP nops -- the walrus in this container caps sync-waits per
    CTRL instruction at 1."""

    def _drain_and_barrier(self, tick_clock, wait_clock):
        nc = self.nc
        lead = nc.sync.nop(nofuse=True)
        wait_clock.add_sem_waits(
            lead.ins, ScopedClock({None: tick_clock.global_clock})
        )
        si = lead.ins.sync_info
        waits = list(si.on_wait) if (si and si.on_wait) else []
        if len(waits) > 1:
            si.on_wait = waits[:1]
            # distribute the remaining waits round-robin across all engine
            # sequencers -- they run in parallel and the all_engine_barrier
            # below joins them, so this is ~5x faster than a serial SP chain
            engs = [nc.sync, nc.vector, nc.scalar, nc.tensor, nc.gpsimd]
            for k, w in enumerate(waits[1:]):
                n = engs[k % len(engs)].nop(nofuse=True)
                nsi = n.ins.sync_info
                if nsi is None:
                    n.ins.sync_info = mybir.SyncInfo(on_wait=[w], on_update=[])
                else:
                    nsi.on_wait = [w]
        nc.sync.drain()
        nc.all_engine_barrier()
        assert self.sems is not None
        popped = nc._tile_sem_poison_stack.pop()
        assert popped is self._sem_poison
        # final barrier elided: the clears run on gpsimd's stream and
        # end-of-execution engine completion already covers them
        nc.clear_and_free_semaphores(list(self.sems.allocated().values()))


def _split_multi_waits(nc):
    """The walrus build in this container caps sync-waits at 1 per
    instruction. Hoist extra waits onto same-engine nops inserted just
    before the offending instruction (engine sequencers are serial, so
    chained single-wait nops are semantically identical)."""
    cur = nc.cur_bb.bb

    def make_nop(engine):
        bi = nc.engines[engine].nop(nofuse=True)
        lst = cur.instructions
        assert lst[-1].name == bi.ins.name
        cur.instructions = lst[:-1]
        return bi.ins

    for f in nc.m.functions:
        for bb in f.blocks:
            insts = bb.instructions
            out = []
            changed = False
            for ins in insts:
                si = ins.sync_info
                waits = list(si.on_wait) if (si and si.on_wait) else []
                if len(waits) > 1:
                    changed = True
                    for w in waits[:-1]:
                        nop = make_nop(ins.engine)
                        nsi = nop.sync_info
                        if nsi is None:
                            nop.sync_info = mybir.SyncInfo(
                                on_wait=[w], on_update=[]
                            )
                        else:
                            nsi.on_wait = [w]
                        out.append(nop)
                    si.on_wait = waits[-1:]
                out.append(ins)
            if changed:
                bb.instructions = out


def _sel_const():
    # SEL[p, j] = 1/B where p//16 == j: contracts o within a (c,o)-group
    # of 96 partitions down to 6 capsules, folding the batch-mean scale.
    sel = np.zeros((KC, CL // 2), dtype=ml_dtypes.bfloat16)
    for p in range(KC):
        sel[p, p // O] = 1.0 / B
    return sel


def build_nc():
    nc = bass.Bass()
    # Inputs land as many small pieces, all triggered in strict consumption
    # order on the sync queue: wx (s-phase, needed first) -> xb (M'-phase)
    # -> w2 (P-mult).  Per-piece completion semaphores let the it0 s-matmuls
    # start as soon as the first 3 chunks arrive instead of waiting for the
    # whole 3.4MB of wx.
    wx_d = nc.declare_dram_parameter("wx", [10, KP, 3, M + B], BF16, isOutput=False)
    xb_d = nc.declare_dram_parameter("xb", [4, 128, 2, KT // 4], BF16, isOutput=False)
    w2_d = nc.declare_dram_parameter("w2", [2, 2, KC, KT // 2], BF16, isOutput=False)
    out_d = nc.declare_dram_parameter("out", [B, M], F32, isOutput=True)
    ident_d = nc.inline_tensor(np.eye(CL, dtype=np.float32), "ident")
    sel_d = nc.inline_tensor(_sel_const(), "sel")

    with _TC(nc) as tc:
        with (
            tc.tile_pool(name="big", bufs=1) as big,
            tc.tile_pool(name="wcp", bufs=1) as wcp,
            tc.tile_pool(name="sm", bufs=2) as sm,
            tc.tile_pool(name="ps_s", bufs=1, space="PSUM") as ps_s,
            tc.tile_pool(name="ps_m", bufs=2, space="PSUM") as ps_m,
            tc.tile_pool(name="ps_t", bufs=2, space="PSUM") as ps_t,
        ):
            # ---- persistent SBUF tensors -------------------------------
            wx_t = [big.tile([KP, CPW, M + B], BF16, tag=f"wx{j}", name=f"wx{j}")
                    for j in range(NW)]
            xb_t = [big.tile([128, 2, KT // 2], BF16, tag=f"xb{j}", name=f"xb{j}")
                    for j in range(2)]
            w2_t = [big.tile([KC, KT], BF16, tag=f"w2{g}", name=f"w2{g}")
                    for g in range(2)]
            ident = big.tile([CL, CL], F32, tag="ident")
            sel = big.tile([KC, CL // 2], BF16, tag="sel")
            bT = [big.tile([CL // 2, R], F32, tag=f"bT{h}", name=f"bT{h}")
                  for h in range(2)]

            nc.scalar.dma_start(ident[:], ident_d[:])
            nc.scalar.dma_start(sel[:], sel_d[:])
            for h in range(10):
                j, hl = divmod(h, 5)
                nc.sync.dma_start(
                    wx_t[j][:, hl * 3:(hl + 1) * 3, :], wx_d[h]
                )
            for q in range(2):
                for h in range(2):
                    nc.sync.dma_start(
                        xb_t[q][:, :, h * 960:(h + 1) * 960], xb_d[2 * q + h]
                    )
            for g in range(2):
                for h in range(2):
                    nc.sync.dma_start(
                        w2_t[g][:, h * (KT // 2):(h + 1) * (KT // 2)], w2_d[g][h]
                    )

            def wt_c(t):   # wt chunk t -> [128, 192] AP
                return wx_t[t // CPW][:, t % CPW, 0:M]

            def xt_c(t, bt):  # [128, 128] lhsT for s-matmul
                return wx_t[t // CPW][:, t % CPW, M + bt * 128:M + (bt + 1) * 128]

            def xb_s(bt, j):  # [128, 480] moving slice for M'-matmul
                q, r0 = divmod(j * JW, KT // 2)
                return xb_t[q][:, bt, r0:r0 + JW]

            def w2_s(g, j):  # [96, 480] W2 slice
                return w2_t[g][:, j * JW:(j + 1) * JW]

            # wc lives in per-half tiles (bf16): separate tiles per capsule
            # half so one half's writes never alias the other half's reads
            # in the range tracker (a shared tile serialized s-matmuls
            # behind the deferred wc multiplies)
            wc_t = [[wcp.tile([KP, CPW, KC], BF16, tag=f"wc{j}g{g}",
                              name=f"wc{j}g{g}")
                     for g in range(2)] for j in range(NW)]

            def wc_c(t, g):
                return wc_t[t // CPW][g][:, t % CPW, :]

            co = dict(o=O)

            def warm(anchor):
                # 1x1 matmul anchored on `anchor` ([1, 1] SBUF AP) -- keeps
                # the PE HAM window busy through DVE/ACT stretches so matmul
                # phases restart at 2.4 GHz.
                wp = ps_t.tile([1, 1], F32, tag="pst", name="warmp")
                nc.tensor.matmul(wp[:], anchor, anchor, start=True, stop=True)


            vT = None
            pending_pe = None   # deferred PE ops from the previous a-phase,
                                # emitted mid-s-loop to avoid PE FIFO stalls
            for it in range(NITER):
                last = it == NITER - 1

                # ---- s-matmul: s[b,(c,o)] accumulated over 40 chunks ---
                # it0: N=192 full-width MMs from wt; later iterations run
                # per capsule-half so half-0's matmuls overlap half-1's
                # routing math on DVE.
                s_ps = [ps_s.tile([128, M], F32, tag=f"s{bt}", name=f"s{bt}")[:]
                        for bt in range(2)]
                if it == 0:
                    # chunk-outer so the matmul consumption order matches the
                    # piecewise DMA arrival order (phase is DMA-paced)
                    for t in range(NK):
                        for bt in range(2):
                            nc.tensor.matmul(
                                s_ps[bt],
                                xt_c(t, bt),
                                wt_c(t),
                                start=(t == 0),
                                stop=(t == NK - 1),
                            )
                else:
                    for g in range(2):
                        for bt in range(2):
                            if g == 0 and bt == 1 and pending_pe is not None:
                                # g=1 transposes of the previous routing pass:
                                # their cT dep is ready by now, and emitting
                                # them here (not before s g0) keeps the PE
                                # queue stall-free
                                pending_pe()
                                pending_pe = None
                            for t in range(NK):
                                nc.tensor.matmul(
                                    s_ps[bt][:, g * KC:(g + 1) * KC],
                                    xt_c(t, bt),
                                    wc_c(t, g),
                                    start=(t == 0),
                                    stop=(t == NK - 1),
                                )

                # ---- squash (split per capsule-half: M'-half-g and the
                # final output DMA only need their own half of v) ----------
                scale = 1.0 / R if it == 0 else 1.0
                CH = CL // 2
                vT = sm.tile([128, 2, M], BF16, tag="vT")
                vOut = (
                    sm.tile([128, 2, M], F32, tag="vOut", name="vOut")
                    if last
                    else None
                )
                for g2 in range(2):
                    for bt in range(2):
                        s_h = s_ps[bt][:, g2 * KC:(g2 + 1) * KC]
                        sq = sm.tile([128, KC], F32, tag="sq", bufs=4)
                        nc.scalar.activation(sq[:], s_h, AF.Square, scale=scale)
                        n2 = sm.tile([128, CH], F32, tag="n2", bufs=4)
                        nc.vector.reduce_sum(
                            n2[:], sq[:].rearrange("p (c o) -> p c o", **co),
                            axis=AX.X,
                        )
                        # sqrt via exp(0.5*ln(.)) -- keeps every activation in
                        # one ACT table set (no ~2.7us table swaps)
                        lnn = sm.tile([128, CH], F32, tag="lnn", bufs=4)
                        nc.scalar.activation(lnn[:], n2[:], AF.Ln)
                        nrm = sm.tile([128, CH], F32, tag="nrm", bufs=4)
                        nc.scalar.activation(nrm[:], lnn[:], AF.Exp, scale=0.5)
                        den = sm.tile([128, CH], F32, tag="den", bufs=4)
                        nc.scalar.activation(den[:], n2[:], AF.Identity, bias=1.0)
                        rden = sm.tile([128, CH], F32, tag="rden", bufs=4)
                        nc.vector.reciprocal(rden[:], den[:])
                        g = sm.tile([128, CH], F32, tag="g", bufs=4)
                        nc.vector.scalar_tensor_tensor(
                            g[:], nrm[:], scale, rden[:],
                            op0=ALU.mult, op1=ALU.mult,
                        )
                        vdst = (vOut if last else vT)[:, bt, g2 * KC:(g2 + 1) * KC]
                        nc.vector.tensor_tensor(
                            vdst.rearrange("p (c o) -> p c o", **co),
                            s_h.rearrange("p (c o) -> p c o", **co),
                            g[:].to_broadcast([128, CH, O]),
                            op=ALU.mult,
                        )
                        if last:
                            nc.sync.dma_start(
                                out_d[bt * 128:(bt + 1) * 128,
                                      g2 * KC:(g2 + 1) * KC],
                                vOut[:, bt, g2 * KC:(g2 + 1) * KC],
                            )
                if last:
                    break

                # ---- a-phase: M' = v^T x, P = W2.*M' ------------------
                # a_ij = SEL.T @ (sum_i P): the o-contraction runs through
                # the SEL stationary and the i-sum through 20 accumulating
                # matmuls on strided rhs views -- no DVE group-reduce.
                # PE FIFO layout (stall-free): [M' g0][SEL g0][M' g1]
                # [transp g0][SEL g1][s g0 ...][transp g1][... s g1]
                pb = sm.tile([KC, 2, KT], BF16, tag="pb")   # P bf16
                # cB planes: s-chunk t at partition p holds r=(p+64*shift)%192
                # with shift = (0,2,1)[t%3]; one broadcast plane per shift
                cB = sm.tile([KP, 2, 3, KC], BF16, tag="cB")
                cT_h = [None, None]

                def route_tail(g, aps):
                    # b-update + softmax over r for this half; no
                    # max-subtraction -- b_ij is a 2-step sum of batch-mean
                    # agreements, bounded well inside exp's f32 range
                    if it == 0:
                        nc.scalar.copy(bT[g][:], aps[:])
                    else:
                        nc.vector.tensor_tensor(
                            bT[g][:], bT[g][:], aps[:], op=ALU.add
                        )
                    eT = sm.tile([CL // 2, R], F32, tag="eT")
                    ssum = sm.tile([CL // 2, 1], F32, tag="ssum")
                    nc.scalar.activation(
                        eT[:], bT[g][:], AF.Exp, accum_out=ssum[:]
                    )
                    rs = sm.tile([CL // 2, 1], F32, tag="rs")
                    nc.vector.reciprocal(rs[:], ssum[:])
                    # cT extended to 256 cols (wraps the first 64) so every
                    # cB plane is a plain 128-wide transpose window
                    cT = sm.tile([CL // 2, R + 64], F32, tag="cT")
                    nc.scalar.activation(cT[:, 0:R], eT[:], AF.Copy, scale=rs[:])
                    nc.scalar.activation(
                        cT[:, R:R + 64], eT[:, 0:64], AF.Copy, scale=rs[:]
                    )
                    cT_h[g] = cT

                def transp_cb_wc(g):
                    cT = cT_h[g]
                    ident6 = ident[0:CL // 2, 0:CL // 2]
                    for sh in range(3):
                        cps = ps_t.tile([KP, CL // 2], F32, tag="pst",
                                        name=f"cps{sh}")
                        nc.tensor.transpose(
                            cps[:], cT[:, 64 * sh:64 * sh + 128], ident6
                        )
                        nc.scalar.copy(
                            cB[:, g, sh, :].rearrange("p (c o) -> p c o", **co),
                            cps[:].to_broadcast([KP, CL // 2, O]),
                        )
                    # wc-half = cB-plane .* wt-half  (bf16 step-1 -> 2x);
                    # chunk t uses shift (0,2,1)[t%3]
                    SMAP = (0, 2, 1)
                    for j in range(NW):
                        wxv = wx_t[j][:].rearrange("p (u s) f -> p s u f", s=3)
                        wcv = wc_t[j][g][:].rearrange(
                            "p (u s) m -> p s u m", s=3
                        )
                        for s in range(3):
                            nc.vector.tensor_tensor(
                                wcv[:, s].rearrange(
                                    "p u (c o) -> p u c o", **co
                                ),
                                wxv[:, s, :, g * KC:(g + 1) * KC].rearrange(
                                    "p u (c o) -> p u c o", **co
                                ),
                                cB[:, g, SMAP[s], :].rearrange(
                                    "p (c o) -> p c o", **co
                                ).unsqueeze(1).broadcast_to(
                                    [KP, CPW // 3, CL // 2, O]
                                ),
                                op=ALU.mult,
                            )

                for g in range(2):
                    for jp in range(NJ // 2):
                        mps = ps_m.tile([KC, 2, 512], F32, tag="mps")
                        for q in range(2):        # two psum banks per tile
                            for bt in range(2):
                                nc.tensor.matmul(
                                    mps[:, q, 0:JW],
                                    vT[:, bt, g * KC:(g + 1) * KC],
                                    xb_s(bt, 2 * jp + q),
                                    start=(bt == 0),
                                    stop=(bt == 1),
                                )
                        mb = sm.tile([KC, 2 * JW], BF16, tag="mb")
                        nc.scalar.copy(
                            mb[:].rearrange('p (q j) -> p q j', q=2),
                            mps[:, :, 0:JW],
                        )
                        nc.vector.tensor_tensor(
                            pb[:, g, 2 * jp * JW:(2 * jp + 2) * JW], mb[:],
                            w2_t[g][:, 2 * jp * JW:(2 * jp + 2) * JW],
                            op=ALU.mult,
                        )
                    if g == 1:
                        transp_cb_wc(0)
                    # i-reduction: k is laid out i-major (k = i*192 + r), so
                    # the 20-term i-sum is 5 unit-stride full-rate DVE adds
                    # (a grouped reduce_sum runs ~10x slower)
                    with nc.allow_low_precision("a_ij steers routing only"):
                        t1 = sm.tile([KC, KT // 2], BF16, tag="t1")
                        nc.vector.tensor_tensor(
                            t1[:], pb[:, g, 0:1920], pb[:, g, 1920:3840],
                            op=ALU.add,
                        )
                        t2 = sm.tile([KC, KT // 4], BF16, tag="t2")
                        nc.vector.tensor_tensor(
                            t2[:], t1[:, 0:960], t1[:, 960:1920], op=ALU.add
                        )
                        t3 = sm.tile([KC, 2 * R], BF16, tag="t3")
                        nc.vector.tensor_tensor(
                            t3[:], t2[:, 0:384], t2[:, 384:768], op=ALU.add
                        )
                        t4 = sm.tile([KC, R], BF16, tag="t4")
                        nc.vector.tensor_tensor(
                            t4[:], t3[:, 0:192], t3[:, 192:384], op=ALU.add
                        )
                        qTg = sm.tile([KC, R], BF16, tag="qTg")
                        nc.vector.tensor_tensor(
                            qTg[:], t4[:], t2[:, 768:960], op=ALU.add
                        )
                    aps = ps_t.tile([CL // 2, R], F32, tag="pst", name="aps")
                    nc.tensor.matmul(
                        aps[:], sel[:], qTg[:], start=True, stop=True
                    )
                    route_tail(g, aps)
                pending_pe = lambda g=1: transp_cb_wc(g)
    _split_multi_waits(nc)
    return nc


def prep_inputs(x, W, core):
    """Host-side shard prep for one core -> dict of bf16 arrays."""
    bf = ml_dtypes.bfloat16
    cs = core * CL
    Ws = W[:, cs:cs + CL]
    # s-phase k-order: k' = i*192 + r (i-major); chunk t spans k' = t*128+p
    xflat = np.ascontiguousarray(x.transpose(0, 2, 1)).reshape(B, KT)
    xt = xflat.T.reshape(NK, KP, B).transpose(1, 0, 2)       # [128, 30, 256]
    wt = (
        np.ascontiguousarray(Ws.transpose(3, 0, 1, 2))
        .reshape(KT, M)
        .reshape(NK, KP, M)
        .transpose(1, 0, 2)                                   # [128, 30, 192]
    )
    # xb[pb, bt, k]: k = i*192 + r  (k-layout B, i-major so the kernel's
    # i-reduction is unit-stride)
    xb = (
        np.ascontiguousarray(x.transpose(0, 2, 1))
        .reshape(2, 128, KT)
        .transpose(1, 0, 2)
    )
    # w2[(c,o) % 96, g, k]: rows (c,o), k-layout B i-major
    w2 = (
        Ws.transpose(1, 2, 3, 0)
        .reshape(2, KC, KT)
        .transpose(1, 0, 2)
    )
    wx = np.concatenate([wt, xt], axis=-1)        # [128, 30, 448]
    wx = wx.reshape(KP, 10, 3, M + B).transpose(1, 0, 2, 3)  # [10,128,3,448]
    xb = xb.reshape(128, 2, 4, KT // 4).transpose(2, 0, 1, 3)  # [4,128,2,960]
    w2 = (
        w2.transpose(1, 0, 2)                      # [2, 96, 3840]
        .reshape(2, KC, 2, KT // 2)
        .transpose(0, 2, 1, 3)                     # [2, 2, 96, 1920]
    )
    return {
        "wx": np.ascontiguousarray(wx).astype(bf),
        "xb": np.ascontiguousarray(xb).astype(bf),
        "w2": np.ascontiguousarray(w2).astype(bf),
    }


_CACHED_NC = None


def kernel(x, W):
    from concourse.bass_utils import run_bass_kernel_spmd

    global _CACHED_NC
    x = np.asarray(x, dtype=np.float32)
    W = np.asarray(W, dtype=np.float32)
    if _CACHED_NC is None:
        _CACHED_NC = build_nc()
    nc = _CACHED_NC
    in_maps = [prep_inputs(x, W, core) for core in range(NCORES)]
    res = run_bass_kernel_spmd(nc, in_maps, list(range(NCORES)))
    v = np.empty((B, C, O), dtype=np.float32)
    for core in range(NCORES):
        v[:, core * CL:(core + 1) * CL, :] = (
            res.results[core]["out"].reshape(B, CL, O)
        )
    return v



# revision 31
# speedup vs baseline: 1.1838x; 1.1838x over previous
"""DigitCaps dynamic-routing kernel for 8 TRN2 NeuronCores.

Algorithm (never materializes u_hat):
  Shard over capsules C=96 -> 12 per core (makes every routing step
  core-local: softmax over R is per-capsule, the batch-mean a_ij needs
  no cross-core reduction -> zero collectives).

  Per core, with K = R*I = 3840 the flattened contraction dim:

  s-phase (k-layout A: k = i*2*96 blocks, chunk t <-> i=t//2, r=(t%2)*96+p):
    s[b,(c,o)]    = sum_k wc[k,(c,o)] * xt[k, b]           (PE, 80 mm)
    wc[k,(c,o)]   = cB[k,(c,o)] * wt[k,(c,o)]              (DVE bf16 2x)
    v = squash(s)                                          (ACT+DVE, small)

  a-phase (k-layout B: k = r*20 + i, (c,o) on partitions):
    M'[(c,o), k]  = sum_b v[b,(c,o)] * x[b, k]             (PE, 32 mm N=512,
                                                            stationary=v reused)
    P = W2 .* M'  (ACT copies psum->bf16, DVE 2x muls)
    Q[(c,o), r]   = sum_i P                                (DVE reduce, step-1)
    a[c, r]       = SEL.T @ Q  (SEL = delta(c)/B)          (PE, tiny)
    b_ij += a; c_ij = softmax_r(b_ij)  (two [6,192] half-tiles)

  Matmul inputs bf16 (measured ~3.8e-3 global rel err vs f32 reference),
  routing/softmax/squash math in f32.
"""

import numpy as np
import ml_dtypes

import concourse.bass as bass
import concourse.mybir as mybir
from concourse import tile
from concourse.vector_clock import ScopedClock

B, R, C, O, I = 256, 192, 96, 16, 20
NCORES = 8
CL = C // NCORES          # 12 capsules per core
M = CL * O                # 192 = (c,o) free dim per core
KC = R // 2               # 96 = (c,o) rows per half (a-phase partition dim)
KT = R * I                # 3840 total contraction
KP = 128                  # s-phase k-chunk depth (full PE rows)
NK = KT // KP             # 30 s-phase k-chunks
NW = 2                    # wx SBUF tiles (15 chunks each)
CPW = NK // NW            # 15
NJ = 8                    # a-phase moving slices (3840 = 8 * 480)
JW = KT // NJ             # 480
NITER = 3

F32 = mybir.dt.float32
BF16 = mybir.dt.bfloat16
AF = mybir.ActivationFunctionType
ALU = mybir.AluOpType
AX = mybir.AxisListType


class _TC(tile.TileContext):
    """TileContext whose exit drain splits its semaphore waits across
    chained SP nops -- the walrus in this container caps sync-waits per
    CTRL instruction at 1."""

    def _drain_and_barrier(self, tick_clock, wait_clock):
        nc = self.nc
        lead = nc.sync.nop(nofuse=True)
        wait_clock.add_sem_waits(
            lead.ins, ScopedClock({None: tick_clock.global_clock})
        )
        si = lead.ins.sync_info
        waits = list(si.on_wait) if (si and si.on_wait) else []
        if len(waits) > 1:
            si.on_wait = waits[:1]
            # distribute the remaining waits round-robin across all engine
            # sequencers -- they run in parallel and the all_engine_barrier
            # below joins them, so this is ~5x faster than a serial SP chain
            engs = [nc.sync, nc.vector, nc.scalar, nc.tensor, nc.gpsimd]
            for k, w in enumerate(waits[1:]):
                n = engs[k % len(engs)].nop(nofuse=True)
                nsi = n.ins.sync_info
                if nsi is None:
                    n.ins.sync_info = mybir.SyncInfo(on_wait=[w], on_update=[])
                else:
                    nsi.on_wait = [w]
        nc.sync.drain()
        nc.all_engine_barrier()
        assert self.sems is not None
        popped = nc._tile_sem_poison_stack.pop()
        assert popped is self._sem_poison
        # final barrier elided: the clears run on gpsimd's stream and
        # end-of-execution engine completion already covers them
        nc.clear_and_free_semaphores(list(self.sems.allocated().values()))


def _split_multi_waits(nc):
    """The walrus build in this container caps sync-waits at 1 per
    instruction. Hoist extra waits onto same-engine nops inserted just
    before the offending instruction (engine sequencers are serial, so
    chained single-wait nops are semantically identical)."""
    cur = nc.cur_bb.bb

    def make_nop(engine):
        bi = nc.engines[engine].nop(nofuse=True)
        lst = cur.instructions
        assert lst[-1].name == bi.ins.name
        cur.instructions = lst[:-1]
        return bi.ins

    for f in nc.m.functions:
        for bb in f.blocks:
            insts = bb.instructions
            out = []
            changed = False
            for ins in insts:
                si = ins.sync_info
                waits = list(si.on_wait) if (si and si.on_wait) else []
                if len(waits) > 1:
                    changed = True
                    for w in waits[:-1]:
                        nop = make_nop(ins.engine)
                        nsi = nop.sync_info
                        if nsi is None:
                            nop.sync_info = mybir.SyncInfo(
                                on_wait=[w], on_update=[]
                            )
                        else:
                            nsi.on_wait = [w]
                        out.append(nop)
                    si.on_wait = waits[-1:]
                out.append(ins)
            if changed:
                bb.instructions = out


def _sel_const():
    # SEL[p, j] = 1/B where p//16 == j: contracts o within a (c,o)-group
    # of 96 partitions down to 6 capsules, folding the batch-mean scale.
    sel = np.zeros((KC, CL // 2), dtype=ml_dtypes.bfloat16)
    for p in range(KC):
        sel[p, p // O] = 1.0 / B
    return sel


def build_nc():
    nc = bass.Bass()
    # Inputs land as many small pieces, all triggered in strict consumption
    # order on the sync queue: wx (s-phase, needed first) -> xb (M'-phase)
    # -> w2 (P-mult).  Per-piece completion semaphores let the it0 s-matmuls
    # start as soon as the first 3 chunks arrive instead of waiting for the
    # whole 3.4MB of wx.
    wx_d = nc.declare_dram_parameter("wx", [10, KP, 3, M + B], BF16, isOutput=False)
    xb_d = nc.declare_dram_parameter("xb", [4, 128, 2, KT // 4], BF16, isOutput=False)
    w2_d = nc.declare_dram_parameter("w2", [2, 2, KC, KT // 2], BF16, isOutput=False)
    out_d = nc.declare_dram_parameter("out", [B, M], F32, isOutput=True)
    ident_d = nc.inline_tensor(np.eye(CL, dtype=np.float32), "ident")
    sel_d = nc.inline_tensor(_sel_const(), "sel")

    with _TC(nc) as tc:
        with (
            tc.tile_pool(name="big", bufs=1) as big,
            tc.tile_pool(name="wcp", bufs=1) as wcp,
            tc.tile_pool(name="sm", bufs=2) as sm,
            tc.tile_pool(name="ps_s", bufs=1, space="PSUM") as ps_s,
            tc.tile_pool(name="ps_m", bufs=2, space="PSUM") as ps_m,
            tc.tile_pool(name="ps_t", bufs=2, space="PSUM") as ps_t,
        ):
            # ---- persistent SBUF tensors -------------------------------
            # wx/wc are class-major: chunk t lives at [s=(t%15)%3, u=(t%15)//3]
            # so per-class wc multiplies write contiguous slabs
            wx_t = [big.tile([KP, 3, CPW // 3, M + B], BF16, tag=f"wx{j}",
                             name=f"wx{j}")
                    for j in range(NW)]
            xb_t = [big.tile([128, 2, KT // 2], BF16, tag=f"xb{j}", name=f"xb{j}")
                    for j in range(2)]
            w2_t = [big.tile([KC, KT], BF16, tag=f"w2{g}", name=f"w2{g}")
                    for g in range(2)]
            ident = big.tile([CL, CL], F32, tag="ident")
            sel = big.tile([KC, CL // 2], BF16, tag="sel")
            bT = [big.tile([CL // 2, R], F32, tag=f"bT{h}", name=f"bT{h}")
                  for h in range(2)]

            nc.scalar.dma_start(ident[:], ident_d[:])
            nc.scalar.dma_start(sel[:], sel_d[:])
            for h in range(10):
                j, hl = divmod(h, 5)
                nc.sync.dma_start(wx_t[j][:, :, hl, :], wx_d[h])
            for q in range(2):
                for h in range(2):
                    nc.sync.dma_start(
                        xb_t[q][:, :, h * 960:(h + 1) * 960], xb_d[2 * q + h]
                    )
            for g in range(2):
                for h in range(2):
                    nc.sync.dma_start(
                        w2_t[g][:, h * (KT // 2):(h + 1) * (KT // 2)], w2_d[g][h]
                    )

            def wt_c(t):   # wt chunk t -> [128, 192] AP
                tl = t % CPW
                return wx_t[t // CPW][:, tl % 3, tl // 3, 0:M]

            def xt_c(t, bt):  # [128, 128] lhsT for s-matmul
                tl = t % CPW
                return wx_t[t // CPW][:, tl % 3, tl // 3,
                                      M + bt * 128:M + (bt + 1) * 128]

            def xb_s(bt, j):  # [128, 480] moving slice for M'-matmul
                q, r0 = divmod(j * JW, KT // 2)
                return xb_t[q][:, bt, r0:r0 + JW]

            def w2_s(g, j):  # [96, 480] W2 slice
                return w2_t[g][:, j * JW:(j + 1) * JW]

            # wc lives in per-half tiles (bf16): separate tiles per capsule
            # half so one half's writes never alias the other half's reads
            # in the range tracker (a shared tile serialized s-matmuls
            # behind the deferred wc multiplies)
            wc_t = [[wcp.tile([KP, 3, CPW // 3, KC], BF16, tag=f"wc{j}g{g}",
                              name=f"wc{j}g{g}")
                     for g in range(2)] for j in range(NW)]

            def wc_c(t, g):
                tl = t % CPW
                return wc_t[t // CPW][g][:, tl % 3, tl // 3, :]

            co = dict(o=O)

            def warm(anchor):
                # 1x1 matmul anchored on `anchor` ([1, 1] SBUF AP) -- keeps
                # the PE HAM window busy through DVE/ACT stretches so matmul
                # phases restart at 2.4 GHz.
                wp = ps_t.tile([1, 1], F32, tag="pst", name="warmp")
                nc.tensor.matmul(wp[:], anchor, anchor, start=True, stop=True)


            vT = None
            pending_pe = None   # deferred PE ops from the previous a-phase,
                                # emitted mid-s-loop to avoid PE FIFO stalls
            for it in range(NITER):
                last = it == NITER - 1

                # ---- s-matmul: s[b,(c,o)] accumulated over 40 chunks ---
                # it0: N=192 full-width MMs from wt; later iterations run
                # per capsule-half so half-0's matmuls overlap half-1's
                # routing math on DVE.
                s_ps = [ps_s.tile([128, M], F32, tag=f"s{bt}", name=f"s{bt}")[:]
                        for bt in range(2)]
                if it == 0:
                    # chunk-outer so the matmul consumption order matches the
                    # piecewise DMA arrival order (phase is DMA-paced)
                    for t in range(NK):
                        for bt in range(2):
                            nc.tensor.matmul(
                                s_ps[bt],
                                xt_c(t, bt),
                                wt_c(t),
                                start=(t == 0),
                                stop=(t == NK - 1),
                            )
                else:
                    for g in range(2):
                        for bt in range(2):
                            if g == 0 and bt == 1 and pending_pe is not None:
                                # g=1 transposes of the previous routing pass:
                                # their cT dep is ready by now, and emitting
                                # them here (not before s g0) keeps the PE
                                # queue stall-free
                                pending_pe()
                                pending_pe = None
                            for t in range(NK):
                                nc.tensor.matmul(
                                    s_ps[bt][:, g * KC:(g + 1) * KC],
                                    xt_c(t, bt),
                                    wc_c(t, g),
                                    start=(t == 0),
                                    stop=(t == NK - 1),
                                )

                # ---- squash (split per capsule-half: M'-half-g and the
                # final output DMA only need their own half of v) ----------
                scale = 1.0 / R if it == 0 else 1.0
                CH = CL // 2
                vT = sm.tile([128, 2, M], BF16, tag="vT")
                vOut = (
                    sm.tile([128, 2, M], F32, tag="vOut", name="vOut")
                    if last
                    else None
                )
                for g2 in range(2):
                    for bt in range(2):
                        s_h = s_ps[bt][:, g2 * KC:(g2 + 1) * KC]
                        sq = sm.tile([128, KC], F32, tag="sq", bufs=4)
                        nc.scalar.activation(sq[:], s_h, AF.Square, scale=scale)
                        n2 = sm.tile([128, CH], F32, tag="n2", bufs=4)
                        nc.vector.reduce_sum(
                            n2[:], sq[:].rearrange("p (c o) -> p c o", **co),
                            axis=AX.X,
                        )
                        # sqrt via exp(0.5*ln(.)) -- keeps every activation in
                        # one ACT table set (no ~2.7us table swaps)
                        lnn = sm.tile([128, CH], F32, tag="lnn", bufs=4)
                        nc.scalar.activation(lnn[:], n2[:], AF.Ln)
                        nrm = sm.tile([128, CH], F32, tag="nrm", bufs=4)
                        nc.scalar.activation(nrm[:], lnn[:], AF.Exp, scale=0.5)
                        den = sm.tile([128, CH], F32, tag="den", bufs=4)
                        nc.scalar.activation(den[:], n2[:], AF.Identity, bias=1.0)
                        rden = sm.tile([128, CH], F32, tag="rden", bufs=4)
                        nc.vector.reciprocal(rden[:], den[:])
                        g = sm.tile([128, CH], F32, tag="g", bufs=4)
                        nc.vector.scalar_tensor_tensor(
                            g[:], nrm[:], scale, rden[:],
                            op0=ALU.mult, op1=ALU.mult,
                        )
                        vdst = (vOut if last else vT)[:, bt, g2 * KC:(g2 + 1) * KC]
                        nc.vector.tensor_tensor(
                            vdst.rearrange("p (c o) -> p c o", **co),
                            s_h.rearrange("p (c o) -> p c o", **co),
                            g[:].to_broadcast([128, CH, O]),
                            op=ALU.mult,
                        )
                        if last:
                            nc.sync.dma_start(
                                out_d[bt * 128:(bt + 1) * 128,
                                      g2 * KC:(g2 + 1) * KC],
                                vOut[:, bt, g2 * KC:(g2 + 1) * KC],
                            )
                if last:
                    break

                # ---- a-phase: M' = v^T x, P = W2.*M' ------------------
                # a_ij = SEL.T @ (sum_i P): the o-contraction runs through
                # the SEL stationary and the i-sum through 20 accumulating
                # matmuls on strided rhs views -- no DVE group-reduce.
                # PE FIFO layout (stall-free): [M' g0][SEL g0][M' g1]
                # [transp g0][SEL g1][s g0 ...][transp g1][... s g1]
                pb = sm.tile([KC, 2, KT], BF16, tag="pb")   # P bf16
                cT_h = [None, None]

                def route_tail(g, aps):
                    # b-update + softmax over r for this half; no
                    # max-subtraction -- b_ij is a 2-step sum of batch-mean
                    # agreements, bounded well inside exp's f32 range
                    if it == 0:
                        nc.scalar.copy(bT[g][:], aps[:])
                    else:
                        nc.vector.tensor_tensor(
                            bT[g][:], bT[g][:], aps[:], op=ALU.add
                        )
                    eT = sm.tile([CL // 2, R], F32, tag="eT")
                    ssum = sm.tile([CL // 2, 1], F32, tag="ssum")
                    nc.scalar.activation(
                        eT[:], bT[g][:], AF.Exp, accum_out=ssum[:]
                    )
                    rs = sm.tile([CL // 2, 1], F32, tag="rs")
                    nc.vector.reciprocal(rs[:], ssum[:])
                    # cT extended to 256 cols (wraps the first 64) so every
                    # cB plane is a plain 128-wide transpose window
                    cT = sm.tile([CL // 2, R + 64], F32, tag="cT")
                    nc.scalar.activation(cT[:, 0:R], eT[:], AF.Copy, scale=rs[:])
                    nc.scalar.activation(
                        cT[:, R:R + 64], eT[:, 0:64], AF.Copy, scale=rs[:]
                    )
                    cT_h[g] = cT

                def transp_cb_wc(g):
                    # c-planes: s-chunk t at partition p holds
                    # r = (p + 64*shift) % 192 with shift = (0,2,1)[t%3];
                    # the wc multiply reads the planes straight from PSUM
                    cT = cT_h[g]
                    ident6 = ident[0:CL // 2, 0:CL // 2]
                    cps = ps_t.tile([KP, 3, CL // 2], F32, tag="pst",
                                    name="cps")
                    for sh in range(3):
                        nc.tensor.transpose(
                            cps[:, sh, :], cT[:, 64 * sh:64 * sh + 128],
                            ident6,
                        )
                    SMAP = (0, 2, 1)
                    for j in range(NW):
                        for s in range(3):
                            nc.vector.tensor_tensor(
                                wc_t[j][g][:, s].rearrange(
                                    "p u (c o) -> p u c o", **co
                                ),
                                wx_t[j][:, s, :, g * KC:(g + 1) * KC]
                                .rearrange("p u (c o) -> p u c o", **co),
                                cps[:, SMAP[s], :]
                                .to_broadcast([KP, CL // 2, O])
                                .unsqueeze(1)
                                .broadcast_to([KP, CPW // 3, CL // 2, O]),
                                op=ALU.mult,
                            )

                for g in range(2):
                    for jp in range(NJ // 2):
                        mps = ps_m.tile([KC, 2, 512], F32, tag="mps")
                        for q in range(2):        # two psum banks per tile
                            for bt in range(2):
                                nc.tensor.matmul(
                                    mps[:, q, 0:JW],
                                    vT[:, bt, g * KC:(g + 1) * KC],
                                    xb_s(bt, 2 * jp + q),
                                    start=(bt == 0),
                                    stop=(bt == 1),
                                )
                        mb = sm.tile([KC, 2 * JW], BF16, tag="mb")
                        nc.scalar.copy(
                            mb[:].rearrange('p (q j) -> p q j', q=2),
                            mps[:, :, 0:JW],
                        )
                        nc.vector.tensor_tensor(
                            pb[:, g, 2 * jp * JW:(2 * jp + 2) * JW], mb[:],
                            w2_t[g][:, 2 * jp * JW:(2 * jp + 2) * JW],
                            op=ALU.mult,
                        )
                    if g == 1:
                        transp_cb_wc(0)
                    # i-reduction: k is laid out i-major (k = i*192 + r), so
                    # the 20-term i-sum is 5 unit-stride full-rate DVE adds
                    # (a grouped reduce_sum runs ~10x slower)
                    with nc.allow_low_precision("a_ij steers routing only"):
                        t1 = sm.tile([KC, KT // 2], BF16, tag="t1")
                        nc.vector.tensor_tensor(
                            t1[:], pb[:, g, 0:1920], pb[:, g, 1920:3840],
                            op=ALU.add,
                        )
                        t2 = sm.tile([KC, KT // 4], BF16, tag="t2")
                        nc.vector.tensor_tensor(
                            t2[:], t1[:, 0:960], t1[:, 960:1920], op=ALU.add
                        )
                        t3 = sm.tile([KC, 2 * R], BF16, tag="t3")
                        nc.vector.tensor_tensor(
                            t3[:], t2[:, 0:384], t2[:, 384:768], op=ALU.add
                        )
                        t4 = sm.tile([KC, R], BF16, tag="t4")
                        nc.vector.tensor_tensor(
                            t4[:], t3[:, 0:192], t3[:, 192:384], op=ALU.add
                        )
                        qTg = sm.tile([KC, R], BF16, tag="qTg")
                        nc.vector.tensor_tensor(
                            qTg[:], t4[:], t2[:, 768:960], op=ALU.add
                        )
                    aps = ps_t.tile([CL // 2, R], F32, tag="pst", name="aps")
                    nc.tensor.matmul(
                        aps[:], sel[:], qTg[:], start=True, stop=True
                    )
                    route_tail(g, aps)
                pending_pe = lambda g=1: transp_cb_wc(g)
    _split_multi_waits(nc)
    return nc


def prep_inputs(x, W, core):
    """Host-side shard prep for one core -> dict of bf16 arrays."""
    bf = ml_dtypes.bfloat16
    cs = core * CL
    Ws = W[:, cs:cs + CL]
    # s-phase k-order: k' = i*192 + r (i-major); chunk t spans k' = t*128+p
    xflat = np.ascontiguousarray(x.transpose(0, 2, 1)).reshape(B, KT)
    xt = xflat.T.reshape(NK, KP, B).transpose(1, 0, 2)       # [128, 30, 256]
    wt = (
        np.ascontiguousarray(Ws.transpose(3, 0, 1, 2))
        .reshape(KT, M)
        .reshape(NK, KP, M)
        .transpose(1, 0, 2)                                   # [128, 30, 192]
    )
    # xb[pb, bt, k]: k = i*192 + r  (k-layout B, i-major so the kernel's
    # i-reduction is unit-stride)
    xb = (
        np.ascontiguousarray(x.transpose(0, 2, 1))
        .reshape(2, 128, KT)
        .transpose(1, 0, 2)
    )
    # w2[(c,o) % 96, g, k]: rows (c,o), k-layout B i-major
    w2 = (
        Ws.transpose(1, 2, 3, 0)
        .reshape(2, KC, KT)
        .transpose(1, 0, 2)
    )
    wx = np.concatenate([wt, xt], axis=-1)        # [128, 30, 448]
    wx = wx.reshape(KP, 10, 3, M + B).transpose(1, 0, 2, 3)  # [10,128,3,448]
    xb = xb.reshape(128, 2, 4, KT // 4).transpose(2, 0, 1, 3)  # [4,128,2,960]
    w2 = (
        w2.transpose(1, 0, 2)                      # [2, 96, 3840]
        .reshape(2, KC, 2, KT // 2)
        .transpose(0, 2, 1, 3)                     # [2, 2, 96, 1920]
    )
    return {
        "wx": np.ascontiguousarray(wx).astype(bf),
        "xb": np.ascontiguousarray(xb).astype(bf),
        "w2": np.ascontiguousarray(w2).astype(bf),
    }


_CACHED_NC = None


def kernel(x, W):
    from concourse.bass_utils import run_bass_kernel_spmd

    global _CACHED_NC
    x = np.asarray(x, dtype=np.float32)
    W = np.asarray(W, dtype=np.float32)
    if _CACHED_NC is None:
        _CACHED_NC = build_nc()
    nc = _CACHED_NC
    in_maps = [prep_inputs(x, W, core) for core in range(NCORES)]
    res = run_bass_kernel_spmd(nc, in_maps, list(range(NCORES)))
    v = np.empty((B, C, O), dtype=np.float32)
    for core in range(NCORES):
        v[:, core * CL:(core + 1) * CL, :] = (
            res.results[core]["out"].reshape(B, CL, O)
        )
    return v



# revision 32
# speedup vs baseline: 1.1983x; 1.0122x over previous
"""DigitCaps dynamic-routing kernel for 8 TRN2 NeuronCores.

Algorithm (never materializes u_hat):
  Shard over capsules C=96 -> 12 per core (makes every routing step
  core-local: softmax over R is per-capsule, the batch-mean a_ij needs
  no cross-core reduction -> zero collectives).

  Per core, with K = R*I = 3840 the flattened contraction dim:

  s-phase (k-layout A: k = i*2*96 blocks, chunk t <-> i=t//2, r=(t%2)*96+p):
    s[b,(c,o)]    = sum_k wc[k,(c,o)] * xt[k, b]           (PE, 80 mm)
    wc[k,(c,o)]   = cB[k,(c,o)] * wt[k,(c,o)]              (DVE bf16 2x)
    v = squash(s)                                          (ACT+DVE, small)

  a-phase (k-layout B: k = r*20 + i, (c,o) on partitions):
    M'[(c,o), k]  = sum_b v[b,(c,o)] * x[b, k]             (PE, 32 mm N=512,
                                                            stationary=v reused)
    P = W2 .* M'  (ACT copies psum->bf16, DVE 2x muls)
    Q[(c,o), r]   = sum_i P                                (DVE reduce, step-1)
    a[c, r]       = SEL.T @ Q  (SEL = delta(c)/B)          (PE, tiny)
    b_ij += a; c_ij = softmax_r(b_ij)  (two [6,192] half-tiles)

  Matmul inputs bf16 (measured ~3.8e-3 global rel err vs f32 reference),
  routing/softmax/squash math in f32.
"""

import numpy as np
import ml_dtypes

import concourse.bass as bass
import concourse.mybir as mybir
from concourse import tile
from concourse.vector_clock import ScopedClock

B, R, C, O, I = 256, 192, 96, 16, 20
NCORES = 8
CL = C // NCORES          # 12 capsules per core
M = CL * O                # 192 = (c,o) free dim per core
KC = R // 2               # 96 = (c,o) rows per half (a-phase partition dim)
KT = R * I                # 3840 total contraction
KP = 128                  # s-phase k-chunk depth (full PE rows)
NK = KT // KP             # 30 s-phase k-chunks
NW = 2                    # wx SBUF tiles (15 chunks each)
CPW = NK // NW            # 15
NJ = 8                    # a-phase moving slices (3840 = 8 * 480)
JW = KT // NJ             # 480
NITER = 3

F32 = mybir.dt.float32
BF16 = mybir.dt.bfloat16
AF = mybir.ActivationFunctionType
ALU = mybir.AluOpType
AX = mybir.AxisListType


class _TC(tile.TileContext):
    """TileContext whose exit drain splits its semaphore waits across
    chained SP nops -- the walrus in this container caps sync-waits per
    CTRL instruction at 1."""

    def _drain_and_barrier(self, tick_clock, wait_clock):
        nc = self.nc
        lead = nc.sync.nop(nofuse=True)
        wait_clock.add_sem_waits(
            lead.ins, ScopedClock({None: tick_clock.global_clock})
        )
        si = lead.ins.sync_info
        waits = list(si.on_wait) if (si and si.on_wait) else []
        if len(waits) > 1:
            si.on_wait = waits[:1]
            # distribute the remaining waits round-robin across all engine
            # sequencers -- they run in parallel and the all_engine_barrier
            # below joins them, so this is ~5x faster than a serial SP chain
            engs = [nc.sync, nc.vector, nc.scalar, nc.tensor, nc.gpsimd]
            for k, w in enumerate(waits[1:]):
                n = engs[k % len(engs)].nop(nofuse=True)
                nsi = n.ins.sync_info
                if nsi is None:
                    n.ins.sync_info = mybir.SyncInfo(on_wait=[w], on_update=[])
                else:
                    nsi.on_wait = [w]
        nc.sync.drain()
        nc.all_engine_barrier()
        assert self.sems is not None
        popped = nc._tile_sem_poison_stack.pop()
        assert popped is self._sem_poison
        # final barrier elided: the clears run on gpsimd's stream and
        # end-of-execution engine completion already covers them
        nc.clear_and_free_semaphores(list(self.sems.allocated().values()))


def _split_multi_waits(nc):
    """The walrus build in this container caps sync-waits at 1 per
    instruction. Hoist extra waits onto same-engine nops inserted just
    before the offending instruction (engine sequencers are serial, so
    chained single-wait nops are semantically identical)."""
    cur = nc.cur_bb.bb

    def make_nop(engine):
        bi = nc.engines[engine].nop(nofuse=True)
        lst = cur.instructions
        assert lst[-1].name == bi.ins.name
        cur.instructions = lst[:-1]
        return bi.ins

    for f in nc.m.functions:
        for bb in f.blocks:
            insts = bb.instructions
            out = []
            changed = False
            for ins in insts:
                si = ins.sync_info
                waits = list(si.on_wait) if (si and si.on_wait) else []
                if len(waits) > 1:
                    changed = True
                    for w in waits[:-1]:
                        nop = make_nop(ins.engine)
                        nsi = nop.sync_info
                        if nsi is None:
                            nop.sync_info = mybir.SyncInfo(
                                on_wait=[w], on_update=[]
                            )
                        else:
                            nsi.on_wait = [w]
                        out.append(nop)
                    si.on_wait = waits[-1:]
                out.append(ins)
            if changed:
                bb.instructions = out


def _sel_const():
    # SEL[p, j] = 1/B where p//16 == j: contracts o within a (c,o)-group
    # of 96 partitions down to 6 capsules, folding the batch-mean scale.
    sel = np.zeros((KC, CL // 2), dtype=ml_dtypes.bfloat16)
    for p in range(KC):
        sel[p, p // O] = 1.0 / B
    return sel


def build_nc():
    nc = bass.Bass()
    # Inputs land as many small pieces, all triggered in strict consumption
    # order on the sync queue: wx (s-phase, needed first) -> xb (M'-phase)
    # -> w2 (P-mult).  Per-piece completion semaphores let the it0 s-matmuls
    # start as soon as the first 3 chunks arrive instead of waiting for the
    # whole 3.4MB of wx.
    wx_d = nc.declare_dram_parameter("wx", [10, KP, 3, M + B], BF16, isOutput=False)
    xb_d = nc.declare_dram_parameter("xb", [4, 128, 2, KT // 4], BF16, isOutput=False)
    w2_d = nc.declare_dram_parameter("w2", [2, 2, KC, KT // 2], BF16, isOutput=False)
    out_d = nc.declare_dram_parameter("out", [B, M], F32, isOutput=True)
    ident_d = nc.inline_tensor(np.eye(CL, dtype=np.float32), "ident")
    sel_d = nc.inline_tensor(_sel_const(), "sel")

    with _TC(nc) as tc:
        with (
            tc.tile_pool(name="big", bufs=1) as big,
            tc.tile_pool(name="wcp", bufs=1) as wcp,
            tc.tile_pool(name="sm", bufs=2) as sm,
            tc.tile_pool(name="ps_s", bufs=1, space="PSUM") as ps_s,
            tc.tile_pool(name="ps_m", bufs=2, space="PSUM") as ps_m,
            tc.tile_pool(name="ps_t", bufs=2, space="PSUM") as ps_t,
        ):
            # ---- persistent SBUF tensors -------------------------------
            # wx/wc are class-major: chunk t lives at [s=(t%15)%3, u=(t%15)//3]
            # so per-class wc multiplies write contiguous slabs
            wx_t = [big.tile([KP, 3, CPW // 3, M + B], BF16, tag=f"wx{j}",
                             name=f"wx{j}")
                    for j in range(NW)]
            xb_t = [big.tile([128, 2, KT // 2], BF16, tag=f"xb{j}", name=f"xb{j}")
                    for j in range(2)]
            w2_t = [big.tile([KC, KT], BF16, tag=f"w2{g}", name=f"w2{g}")
                    for g in range(2)]
            ident = big.tile([CL, CL], F32, tag="ident")
            sel = big.tile([KC, CL // 2], BF16, tag="sel")
            bT = [big.tile([CL // 2, R], F32, tag=f"bT{h}", name=f"bT{h}")
                  for h in range(2)]

            nc.scalar.dma_start(ident[:], ident_d[:])
            nc.scalar.dma_start(sel[:], sel_d[:])
            for h in range(10):
                j, hl = divmod(h, 5)
                nc.sync.dma_start(wx_t[j][:, :, hl, :], wx_d[h])
            for q in range(2):
                for h in range(2):
                    nc.sync.dma_start(
                        xb_t[q][:, :, h * 960:(h + 1) * 960], xb_d[2 * q + h]
                    )
            for g in range(2):
                for h in range(2):
                    nc.sync.dma_start(
                        w2_t[g][:, h * (KT // 2):(h + 1) * (KT // 2)], w2_d[g][h]
                    )

            def wt_c(t):   # wt chunk t -> [128, 192] AP
                tl = t % CPW
                return wx_t[t // CPW][:, tl % 3, tl // 3, 0:M]

            def xt_c(t, bt):  # [128, 128] lhsT for s-matmul
                tl = t % CPW
                return wx_t[t // CPW][:, tl % 3, tl // 3,
                                      M + bt * 128:M + (bt + 1) * 128]

            def xb_s(bt, j):  # [128, 480] moving slice for M'-matmul
                q, r0 = divmod(j * JW, KT // 2)
                return xb_t[q][:, bt, r0:r0 + JW]

            def w2_s(g, j):  # [96, 480] W2 slice
                return w2_t[g][:, j * JW:(j + 1) * JW]

            # wc lives in per-half tiles (bf16): separate tiles per capsule
            # half so one half's writes never alias the other half's reads
            # in the range tracker (a shared tile serialized s-matmuls
            # behind the deferred wc multiplies)
            wc_t = [[wcp.tile([KP, 3, CPW // 3, KC], BF16, tag=f"wc{j}g{g}",
                              name=f"wc{j}g{g}")
                     for g in range(2)] for j in range(NW)]

            def wc_c(t, g):
                tl = t % CPW
                return wc_t[t // CPW][g][:, tl % 3, tl // 3, :]

            co = dict(o=O)

            def warm(anchor):
                # 1x1 matmul anchored on `anchor` ([1, 1] SBUF AP) -- keeps
                # the PE HAM window busy through DVE/ACT stretches so matmul
                # phases restart at 2.4 GHz.
                wp = ps_t.tile([1, 1], F32, tag="pst", name="warmp")
                nc.tensor.matmul(wp[:], anchor, anchor, start=True, stop=True)


            vT = None
            pending_pe = None   # deferred PE ops from the previous a-phase,
                                # emitted mid-s-loop to avoid PE FIFO stalls
            for it in range(NITER):
                last = it == NITER - 1

                # ---- s-matmul: s[b,(c,o)] accumulated over 40 chunks ---
                # it0: N=192 full-width MMs from wt; later iterations run
                # per capsule-half so half-0's matmuls overlap half-1's
                # routing math on DVE.
                s_ps = [ps_s.tile([128, M], F32, tag=f"s{bt}", name=f"s{bt}")[:]
                        for bt in range(2)]
                if it == 0:
                    # chunk-outer so the matmul consumption order matches the
                    # piecewise DMA arrival order (phase is DMA-paced)
                    for t in range(NK):
                        for bt in range(2):
                            nc.tensor.matmul(
                                s_ps[bt],
                                xt_c(t, bt),
                                wt_c(t),
                                start=(t == 0),
                                stop=(t == NK - 1),
                            )
                else:
                    for g in range(2):
                        for bt in range(2):
                            if g == 0 and bt == 1 and pending_pe is not None:
                                # g=1 transposes of the previous routing pass:
                                # their cT dep is ready by now, and emitting
                                # them here (not before s g0) keeps the PE
                                # queue stall-free
                                pending_pe()
                                pending_pe = None
                            for t in range(NK):
                                nc.tensor.matmul(
                                    s_ps[bt][:, g * KC:(g + 1) * KC],
                                    xt_c(t, bt),
                                    wc_c(t, g),
                                    start=(t == 0),
                                    stop=(t == NK - 1),
                                )

                # ---- squash (split per capsule-half: M'-half-g and the
                # final output DMA only need their own half of v) ----------
                scale = 1.0 / R if it == 0 else 1.0
                CH = CL // 2
                vT = sm.tile([128, 2, M], BF16, tag="vT")
                vOut = (
                    sm.tile([128, 2, M], F32, tag="vOut", name="vOut")
                    if last
                    else None
                )
                for g2 in range(2):
                    for bt in range(2):
                        s_h = s_ps[bt][:, g2 * KC:(g2 + 1) * KC]
                        sq = sm.tile([128, KC], F32, tag="sq", bufs=4)
                        nc.scalar.activation(sq[:], s_h, AF.Square, scale=scale)
                        n2 = sm.tile([128, CH], F32, tag="n2", bufs=4)
                        nc.vector.reduce_sum(
                            n2[:], sq[:].rearrange("p (c o) -> p c o", **co),
                            axis=AX.X,
                        )
                        # sqrt via exp(0.5*ln(.)) -- keeps every activation in
                        # one ACT table set (no ~2.7us table swaps)
                        lnn = sm.tile([128, CH], F32, tag="lnn", bufs=4)
                        nc.scalar.activation(lnn[:], n2[:], AF.Ln)
                        nrm = sm.tile([128, CH], F32, tag="nrm", bufs=4)
                        nc.scalar.activation(nrm[:], lnn[:], AF.Exp, scale=0.5)
                        den = sm.tile([128, CH], F32, tag="den", bufs=4)
                        nc.scalar.activation(den[:], n2[:], AF.Identity, bias=1.0)
                        rden = sm.tile([128, CH], F32, tag="rden", bufs=4)
                        nc.vector.reciprocal(rden[:], den[:])
                        g = sm.tile([128, CH], F32, tag="g", bufs=4)
                        nc.vector.scalar_tensor_tensor(
                            g[:], nrm[:], scale, rden[:],
                            op0=ALU.mult, op1=ALU.mult,
                        )
                        vdst = (vOut if last else vT)[:, bt, g2 * KC:(g2 + 1) * KC]
                        nc.vector.tensor_tensor(
                            vdst.rearrange("p (c o) -> p c o", **co),
                            s_h.rearrange("p (c o) -> p c o", **co),
                            g[:].to_broadcast([128, CH, O]),
                            op=ALU.mult,
                        )
                        if last:
                            nc.sync.dma_start(
                                out_d[bt * 128:(bt + 1) * 128,
                                      g2 * KC:(g2 + 1) * KC],
                                vOut[:, bt, g2 * KC:(g2 + 1) * KC],
                            )
                if last:
                    break

                # ---- a-phase: M' = v^T x, P = W2.*M' ------------------
                # a_ij = SEL.T @ (sum_i P): the o-contraction runs through
                # the SEL stationary and the i-sum through 20 accumulating
                # matmuls on strided rhs views -- no DVE group-reduce.
                # PE FIFO layout (stall-free): [M' g0][SEL g0][M' g1]
                # [transp g0][SEL g1][s g0 ...][transp g1][... s g1]
                pb = sm.tile([KC, 2, KT], BF16, tag="pb")   # P bf16
                cT_h = [None, None]

                def route_tail(g, aps):
                    # b-update + softmax over r for this half; no
                    # max-subtraction -- b_ij is a 2-step sum of batch-mean
                    # agreements, bounded well inside exp's f32 range
                    if it == 0:
                        nc.scalar.copy(bT[g][:], aps[:])
                    else:
                        nc.vector.tensor_tensor(
                            bT[g][:], bT[g][:], aps[:], op=ALU.add
                        )
                    eT = sm.tile([CL // 2, R], F32, tag="eT")
                    ssum = sm.tile([CL // 2, 1], F32, tag="ssum")
                    nc.scalar.activation(
                        eT[:], bT[g][:], AF.Exp, accum_out=ssum[:]
                    )
                    rs = sm.tile([CL // 2, 1], F32, tag="rs")
                    nc.vector.reciprocal(rs[:], ssum[:])
                    # cT extended to 256 cols (wraps the first 64) so every
                    # cB plane is a plain 128-wide transpose window
                    cT = sm.tile([CL // 2, R + 64], F32, tag="cT")
                    nc.scalar.activation(cT[:, 0:R], eT[:], AF.Copy, scale=rs[:])
                    nc.scalar.activation(
                        cT[:, R:R + 64], eT[:, 0:64], AF.Copy, scale=rs[:]
                    )
                    cT_h[g] = cT

                def transp_cb_wc(g):
                    # c-planes: s-chunk t at partition p holds
                    # r = (p + 64*shift) % 192 with shift = (0,2,1)[t%3];
                    # the wc multiply reads the planes straight from PSUM
                    cT = cT_h[g]
                    ident6 = ident[0:CL // 2, 0:CL // 2]
                    cps = ps_t.tile([KP, 3, CL // 2], F32, tag="pst",
                                    name="cps")
                    for sh in range(3):
                        nc.tensor.transpose(
                            cps[:, sh, :], cT[:, 64 * sh:64 * sh + 128],
                            ident6,
                        )
                    # o-broadcast the planes once to bf16 SBUF so the wc
                    # multiplies run all-bf16 (psum-f32 operand halved their
                    # throughput)
                    cBs = sm.tile([KP, 3, KC], BF16, tag="cBs")
                    nc.scalar.copy(
                        cBs[:].rearrange("p s (c o) -> p s c o", **co),
                        cps[:].to_broadcast([KP, 3, CL // 2, O]),
                    )
                    SMAP = (0, 2, 1)
                    for j in range(NW):
                        for s in range(3):
                            nc.vector.tensor_tensor(
                                wc_t[j][g][:, s].rearrange(
                                    "p u (c o) -> p u c o", **co
                                ),
                                wx_t[j][:, s, :, g * KC:(g + 1) * KC]
                                .rearrange("p u (c o) -> p u c o", **co),
                                cBs[:, SMAP[s], :]
                                .rearrange("p (c o) -> p c o", **co)
                                .unsqueeze(1)
                                .broadcast_to([KP, CPW // 3, CL // 2, O]),
                                op=ALU.mult,
                            )

                for g in range(2):
                    for jp in range(NJ // 2):
                        mps = ps_m.tile([KC, 2, 512], F32, tag="mps")
                        for q in range(2):        # two psum banks per tile
                            for bt in range(2):
                                nc.tensor.matmul(
                                    mps[:, q, 0:JW],
                                    vT[:, bt, g * KC:(g + 1) * KC],
                                    xb_s(bt, 2 * jp + q),
                                    start=(bt == 0),
                                    stop=(bt == 1),
                                )
                        mb = sm.tile([KC, 2 * JW], BF16, tag="mb")
                        nc.scalar.copy(
                            mb[:].rearrange('p (q j) -> p q j', q=2),
                            mps[:, :, 0:JW],
                        )
                        nc.vector.tensor_tensor(
                            pb[:, g, 2 * jp * JW:(2 * jp + 2) * JW], mb[:],
                            w2_t[g][:, 2 * jp * JW:(2 * jp + 2) * JW],
                            op=ALU.mult,
                        )
                    if g == 1:
                        transp_cb_wc(0)
                    # i-reduction: k is laid out i-major (k = i*192 + r), so
                    # the 20-term i-sum is 5 unit-stride full-rate DVE adds
                    # (a grouped reduce_sum runs ~10x slower)
                    with nc.allow_low_precision("a_ij steers routing only"):
                        t1 = sm.tile([KC, KT // 2], BF16, tag="t1")
                        nc.vector.tensor_tensor(
                            t1[:], pb[:, g, 0:1920], pb[:, g, 1920:3840],
                            op=ALU.add,
                        )
                        t2 = sm.tile([KC, KT // 4], BF16, tag="t2")
                        nc.vector.tensor_tensor(
                            t2[:], t1[:, 0:960], t1[:, 960:1920], op=ALU.add
                        )
                        t3 = sm.tile([KC, 2 * R], BF16, tag="t3")
                        nc.vector.tensor_tensor(
                            t3[:], t2[:, 0:384], t2[:, 384:768], op=ALU.add
                        )
                        t4 = sm.tile([KC, R], BF16, tag="t4")
                        nc.vector.tensor_tensor(
                            t4[:], t3[:, 0:192], t3[:, 192:384], op=ALU.add
                        )
                        qTg = sm.tile([KC, R], BF16, tag="qTg")
                        nc.vector.tensor_tensor(
                            qTg[:], t4[:], t2[:, 768:960], op=ALU.add
                        )
                    aps = ps_t.tile([CL // 2, R], F32, tag="pst", name="aps")
                    nc.tensor.matmul(
                        aps[:], sel[:], qTg[:], start=True, stop=True
                    )
                    route_tail(g, aps)
                pending_pe = lambda g=1: transp_cb_wc(g)
    _split_multi_waits(nc)
    return nc


def prep_inputs(x, W, core):
    """Host-side shard prep for one core -> dict of bf16 arrays."""
    bf = ml_dtypes.bfloat16
    cs = core * CL
    Ws = W[:, cs:cs + CL]
    # s-phase k-order: k' = i*192 + r (i-major); chunk t spans k' = t*128+p
    xflat = np.ascontiguousarray(x.transpose(0, 2, 1)).reshape(B, KT)
    xt = xflat.T.reshape(NK, KP, B).transpose(1, 0, 2)       # [128, 30, 256]
    wt = (
        np.ascontiguousarray(Ws.transpose(3, 0, 1, 2))
        .reshape(KT, M)
        .reshape(NK, KP, M)
        .transpose(1, 0, 2)                                   # [128, 30, 192]
    )
    # xb[pb, bt, k]: k = i*192 + r  (k-layout B, i-major so the kernel's
    # i-reduction is unit-stride)
    xb = (
        np.ascontiguousarray(x.transpose(0, 2, 1))
        .reshape(2, 128, KT)
        .transpose(1, 0, 2)
    )
    # w2[(c,o) % 96, g, k]: rows (c,o), k-layout B i-major
    w2 = (
        Ws.transpose(1, 2, 3, 0)
        .reshape(2, KC, KT)
        .transpose(1, 0, 2)
    )
    wx = np.concatenate([wt, xt], axis=-1)        # [128, 30, 448]
    wx = wx.reshape(KP, 10, 3, M + B).transpose(1, 0, 2, 3)  # [10,128,3,448]
    xb = xb.reshape(128, 2, 4, KT // 4).transpose(2, 0, 1, 3)  # [4,128,2,960]
    w2 = (
        w2.transpose(1, 0, 2)                      # [2, 96, 3840]
        .reshape(2, KC, 2, KT // 2)
        .transpose(0, 2, 1, 3)                     # [2, 2, 96, 1920]
    )
    return {
        "wx": np.ascontiguousarray(wx).astype(bf),
        "xb": np.ascontiguousarray(xb).astype(bf),
        "w2": np.ascontiguousarray(w2).astype(bf),
    }


_CACHED_NC = None


def kernel(x, W):
    from concourse.bass_utils import run_bass_kernel_spmd

    global _CACHED_NC
    x = np.asarray(x, dtype=np.float32)
    W = np.asarray(W, dtype=np.float32)
    if _CACHED_NC is None:
        _CACHED_NC = build_nc()
    nc = _CACHED_NC
    in_maps = [prep_inputs(x, W, core) for core in range(NCORES)]
    res = run_bass_kernel_spmd(nc, in_maps, list(range(NCORES)))
    v = np.empty((B, C, O), dtype=np.float32)
    for core in range(NCORES):
        v[:, core * CL:(core + 1) * CL, :] = (
            res.results[core]["out"].reshape(B, CL, O)
        )
    return v

